# revision 7
# baseline (speedup 1.0000x reference)
"""Distributed multi-head attention kernel for one TRN2 chip (8 NeuronCores).

Problem: x[2,2048,1024] -> qkv -> 16-head attention -> out proj, f32 I/O.

Sharding: 8 cores = 2 batches x 4 head-groups (4 heads each).
Core c: batch b=c//4, head group g=c%4 (heads 4g..4g+3).
 - x fed host-transposed as xT[b] [1024,2048] bf16 (layout prep on host).
 - w_qkv column-sharded per head group; w_proj row-sharded; proj partials
   ReduceScatter-summed over each 4-core batch group; core 4b+g returns
   output rows [512g:512g+512) of batch b.

Device algorithm (per core), all matmuls bf16 with f32 PSUM accumulate:
 1) qkT = w_qk.T @ xT   [512,2048]  (q pre-scaled by 1/sqrt(dh), bias via ACT)
 2) v   = xT.T @ w_v    [2048,256]  (bias via broadcast + DVE add)
 3) per head-pair, per q-tile: sT = k @ qT (row-tiled 2 heads concurrent),
    p = exp(sT) on ScalarE (PSUM->SBUF bf16), oT += v.T @ p and
    denom += ones.T @ p (col-tiled 2 heads concurrent), normalize oT by
    1/denom during PSUM evict (softmax denominators; no max-subtraction
    needed: |scores*scale| < ~2 so exp is safely in range).
 4) proj partial = o_localT.T @ w_proj_shard + b_proj/4, ReduceScatter(add)
    over the batch group, DMA out [512,1024].
"""

import os
import sys
import types
import numpy as np
import ml_dtypes

import concourse.bass as bass
import concourse.mybir as mybir
import concourse.bacc as bacc
import concourse.tile as tile
from concourse.bass_utils import run_bass_kernel_spmd


def _install_ntff_shim():
    """Provide antenv.axon_hooks (absent from this image's antenv stub) so
    run_bass_kernel_spmd(trace=True) can reach the NTFF profiler in
    libaxon_pjrt.so. Only needed when profiling."""
    if "antenv.axon_hooks" in sys.modules:
        return
    try:
        from trn_agent_boot.trn_boot import _ntff_profile_via_ctypes
        hook = _ntff_profile_via_ctypes("/opt/axon/libaxon_pjrt.so")
    except Exception:
        hook = None
    mod = types.ModuleType("antenv.axon_hooks")
    mod._hook = hook
    mod.get_axon_ntff_profile_hook = lambda: mod._hook
    mod.set_axon_ntff_profile_hook = lambda h: setattr(mod, "_hook", h)
    sys.modules["antenv.axon_hooks"] = mod

BF16 = mybir.dt.bfloat16
F32 = mybir.dt.float32

B, N, D = 2, 2048, 1024
H, DH = 16, 64
SCALE = DH ** -0.5

P = 128                 # partitions
NT = 512                # token free-dim tile
KC = N // P             # 16 k-token chunks
QT = N // NT            # 4 q tiles
DC = D // P             # 8 d_model chunks
HPC = 4                 # heads per core
OF = HPC * DH           # 256 o-features per core

CORE_IDS = list(range(8))
LAST_RESULTS = None     # BassKernelResults of the most recent run


def build_nc():
    nc = bacc.Bacc("TRN2", target_bir_lowering=False, debug=False, num_devices=8)

    xt_ext = nc.dram_tensor("xt", [D, N], BF16, kind="ExternalInput")
    wqk_ext = nc.dram_tensor("wqk", [D, 2 * OF], BF16, kind="ExternalInput")
    wv_ext = nc.dram_tensor("wv", [D, OF], BF16, kind="ExternalInput")
    bqk_ext = nc.dram_tensor("bqk", [P, 4], F32, kind="ExternalInput")
    bv_ext = nc.dram_tensor("bv", [1, OF], F32, kind="ExternalInput")
    wp_ext = nc.dram_tensor("wp", [OF, D], BF16, kind="ExternalInput")
    bp4_ext = nc.dram_tensor("bp4", [1, D], F32, kind="ExternalInput")
    out_ext = nc.dram_tensor("out", [N // 4, D], F32, kind="ExternalOutput")

    rs_in = nc.dram_tensor("rs_in", [N, D], F32)
    rs_out = nc.dram_tensor("rs_out", [N // 4, D], F32)

    with tile.TileContext(nc) as tc:
        with (
            tc.tile_pool(name="xt_pool", bufs=1) as xt_pool,
            tc.tile_pool(name="w_pool", bufs=1) as w_pool,
            tc.tile_pool(name="qk_pool", bufs=1) as qk_pool,
            tc.tile_pool(name="v_pool", bufs=1) as v_pool,
            tc.tile_pool(name="o_pool", bufs=1) as o_pool,
            tc.tile_pool(name="const_pool", bufs=1) as const_pool,
        ):
            # ---- load inputs ----
            xt_t = []
            for k in range(DC):
                t = xt_pool.tile([P, N], BF16, name=f"xt{k}")
                nc.sync.dma_start(t[:], xt_ext[k * P:(k + 1) * P, :])
                xt_t.append(t)
            wqk_t = []
            for k in range(DC):
                t = w_pool.tile([P, 2 * OF], BF16, name=f"wqk{k}")
                nc.sync.dma_start(t[:], wqk_ext[k * P:(k + 1) * P, :])
                wqk_t.append(t)
            wv_t = []
            for k in range(DC):
                t = w_pool.tile([P, OF], BF16, name=f"wv{k}")
                nc.sync.dma_start(t[:], wv_ext[k * P:(k + 1) * P, :])
                wv_t.append(t)
            wp_t = []
            for pair in range(2):
                t = w_pool.tile([P, D], BF16, name=f"wp{pair}")
                nc.sync.dma_start(t[:], wp_ext[pair * P:(pair + 1) * P, :])
                wp_t.append(t)

            bqk_sb = const_pool.tile([P, 4], F32)
            nc.sync.dma_start(bqk_sb[:], bqk_ext[:])
            bv_row = const_pool.tile([1, OF], F32)
            nc.sync.dma_start(bv_row[:], bv_ext[:])
            bp_row = const_pool.tile([1, D], F32)
            nc.sync.dma_start(bp_row[:], bp4_ext[:])

            bv_bc = const_pool.tile([P, OF], F32)
            nc.gpsimd.partition_broadcast(bv_bc[:], bv_row[:])
            bp_bc = const_pool.tile([P, D], F32)
            nc.gpsimd.partition_broadcast(bp_bc[:], bp_row[:])

            ones_col = const_pool.tile([P, 1], BF16)
            nc.vector.memset(ones_col[:], 1.0)

            # ---- phase B: qkT = wqk.T @ xT  -> 4 tiles [128, 2048] bf16 ----
            # m=0: q heads 0-1, m=1: q heads 2-3, m=2: k heads 0-1, m=3: k 2-3
            qk_sb = [qk_pool.tile([P, N], BF16, name=f"qk{m}") for m in range(4)]
            with tc.tile_pool(name="ps_qk", bufs=4, space="PSUM") as ps_qk:
                for m in range(4):
                    for n in range(QT):
                        ps = ps_qk.tile([P, NT], F32)
                        for k in range(DC):
                            nc.tensor.matmul(
                                ps[:], wqk_t[k][:, m * P:(m + 1) * P],
                                xt_t[k][:, n * NT:(n + 1) * NT],
                                start=(k == 0), stop=(k == DC - 1))
                        nc.scalar.activation(
                            qk_sb[m][:, n * NT:(n + 1) * NT], ps[:],
                            mybir.ActivationFunctionType.Identity,
                            bias=bqk_sb[:, m:m + 1],
                            scale=SCALE if m < 2 else 1.0)

            # ---- phase C: v natural [2048, 256] bf16 (16 tiles) ----
            v_sb = [v_pool.tile([P, OF], BF16, name=f"v{t}") for t in range(KC)]
            with tc.tile_pool(name="ps_v", bufs=4, space="PSUM") as ps_v:
                for t in range(KC):
                    ps = ps_v.tile([P, OF], F32)
                    for k in range(DC):
                        nc.tensor.matmul(
                            ps[:], xt_t[k][:, t * P:(t + 1) * P], wv_t[k][:],
                            start=(k == 0), stop=(k == DC - 1))
                    nc.vector.tensor_add(v_sb[t][:], ps[:], bv_bc[:])

            # ---- phase D: attention per head pair ----
            # o_pair[p] rows 0-63 = head 2p, rows 64-127 = head 2p+1
            o_pair = [o_pool.tile([P, N], BF16, name=f"o{p}") for p in range(2)]
            with (
                tc.tile_pool(name="ps_s", bufs=2, space="PSUM") as ps_s,
                tc.tile_pool(name="ps_o", bufs=2, space="PSUM") as ps_o,
                tc.tile_pool(name="ps_d", bufs=2, space="PSUM") as ps_d,
                tc.tile_pool(name="pt_pool", bufs=6) as pt_pool,
                tc.tile_pool(name="nrm_pool", bufs=2) as nrm_pool,
            ):
                for p in range(2):
                    kt = qk_sb[2 + p]
                    qt_ = qk_sb[p]
                    for qt in range(QT):
                        qs = slice(qt * NT, (qt + 1) * NT)
                        po = ps_o.tile([P, NT], F32)
                        pd = ps_d.tile([P, NT], F32)
                        for c in range(KC):
                            cs = slice(c * P, (c + 1) * P)
                            s0 = ps_s.tile([P, NT], F32, name="s0")
                            s1 = ps_s.tile([P, NT], F32, name="s1")
                            nc.tensor.matmul(s0[:], kt[0:64, cs], qt_[0:64, qs],
                                             tile_position=(0, 0),
                                             start=True, stop=True)
                            nc.tensor.matmul(s1[:], kt[64:128, cs], qt_[64:128, qs],
                                             tile_position=(64, 0),
                                             start=True, stop=True)
                            pt0 = pt_pool.tile([P, NT], BF16, name="pt0")
                            pt1 = pt_pool.tile([P, NT], BF16, name="pt1")
                            nc.scalar.activation(
                                pt0[:], s0[:], mybir.ActivationFunctionType.Exp)
                            nc.scalar.activation(
                                pt1[:], s1[:], mybir.ActivationFunctionType.Exp)
                            st, sp = (c == 0), (c == KC - 1)
                            nc.tensor.matmul(po[0:64, :], v_sb[c][:, p * P:p * P + 64],
                                             pt0[:], tile_position=(0, 0),
                                             start=st, stop=sp)
                            nc.tensor.matmul(po[64:128, :],
                                             v_sb[c][:, p * P + 64:(p + 1) * P],
                                             pt1[:], tile_position=(0, 64),
                                             start=st, stop=sp)
                            nc.tensor.matmul(pd[0:1, :], ones_col[:], pt0[:],
                                             tile_position=(0, 0),
                                             start=st, stop=sp)
                            nc.tensor.matmul(pd[64:65, :], ones_col[:], pt1[:],
                                             tile_position=(0, 64),
                                             start=st, stop=sp)
                        # normalize: oT[h] *= 1/denom[h] (per q token).
                        # partition_broadcast only writes from base 0, so
                        # build head B's rows at 0-63 and block-copy to 64-127.
                        rb = nrm_pool.tile([P, NT], F32, name="rb")
                        rb2 = nrm_pool.tile([64, NT], F32, name="rb2")
                        nc.vector.reciprocal(rb[0:1, :], pd[0:1, :])
                        nc.vector.reciprocal(rb2[0:1, :], pd[64:65, :])
                        nc.gpsimd.partition_broadcast(rb[0:64, :], rb[0:1, :])
                        nc.gpsimd.partition_broadcast(rb2[0:64, :], rb2[0:1, :])
                        nc.vector.tensor_copy(rb[64:128, :], rb2[0:64, :])
                        nc.vector.tensor_mul(o_pair[p][:, qs], po[:], rb[:])

            # ---- phase E: proj partial [2048, 1024] f32 -> rs_in ----
            with (
                tc.tile_pool(name="ps_p", bufs=4, space="PSUM") as ps_p,
                tc.tile_pool(name="pr_pool", bufs=4) as pr_pool,
            ):
                for qc in range(KC):
                    for o in range(2):
                        os_ = slice(o * NT, (o + 1) * NT)
                        ps = ps_p.tile([P, NT], F32)
                        for pair in range(2):
                            nc.tensor.matmul(
                                ps[:], o_pair[pair][:, qc * P:(qc + 1) * P],
                                wp_t[pair][:, os_],
                                start=(pair == 0), stop=(pair == 1))
                        pr = pr_pool.tile([P, NT], F32)
                        nc.vector.tensor_add(pr[:], ps[:], bp_bc[:, os_])
                        nc.sync.dma_start(
                            rs_in[qc * P:(qc + 1) * P, os_], pr[:])

            # ---- phase F: ReduceScatter over batch group, write out ----
            nc.gpsimd.collective_compute(
                "ReduceScatter", mybir.AluOpType.add,
                replica_groups=[[0, 1, 2, 3], [4, 5, 6, 7]],
                ins=[rs_in.ap().opt()], outs=[rs_out.ap().opt()])
            nc.sync.dma_start(out_ext[:, :], rs_out[:, :])

    nc.compile()
    return nc


_NC_CACHE = None


def _get_nc():
    global _NC_CACHE
    if _NC_CACHE is None:
        _NC_CACHE = build_nc()
    return _NC_CACHE


def _bf16(a):
    return np.ascontiguousarray(a.astype(ml_dtypes.bfloat16))


def kernel(x, w_qkv, b_qkv, w_proj, b_proj):
    global LAST_RESULTS
    x = np.asarray(x, dtype=np.float32)
    w_qkv = np.asarray(w_qkv, dtype=np.float32)
    b_qkv = np.asarray(b_qkv, dtype=np.float32)
    w_proj = np.asarray(w_proj, dtype=np.float32)
    b_proj = np.asarray(b_proj, dtype=np.float32)

    nc = _get_nc()

    in_maps = []
    for c in CORE_IDS:
        b, g = c // 4, c % 4
        cs = slice(g * OF, (g + 1) * OF)   # feature cols of this head group
        wq = w_qkv[:, 0 * D:1 * D][:, cs]
        wk = w_qkv[:, 1 * D:2 * D][:, cs]
        wv = w_qkv[:, 2 * D:3 * D][:, cs]
        bq = b_qkv[0 * D:1 * D][cs] * SCALE
        bk = b_qkv[1 * D:2 * D][cs]
        bqk = np.concatenate([bq, bk]).reshape(4, P).T.copy()  # [128, 4]
        in_maps.append({
            "xt": _bf16(x[b].T),
            "wqk": _bf16(np.concatenate([wq, wk], axis=1)),
            "wv": _bf16(wv),
            "bqk": np.ascontiguousarray(bqk, dtype=np.float32),
            "bv": np.ascontiguousarray(
                b_qkv[2 * D + g * OF:2 * D + (g + 1) * OF].reshape(1, OF)),
            "wp": _bf16(w_proj[cs, :]),
            "bp4": np.ascontiguousarray((b_proj / 4.0).reshape(1, D),
                                        dtype=np.float32),
        })

    trace = bool(os.environ.get("KERNEL_TRACE"))
    if trace:
        _install_ntff_shim()
    LAST_RESULTS = run_bass_kernel_spmd(
        nc, in_maps, CORE_IDS, trace=trace)

    out = np.empty((B, N, D), dtype=np.float32)
    for c in CORE_IDS:
        b, g = c // 4, c % 4
        out[b, g * (N // 4):(g + 1) * (N // 4), :] = LAST_RESULTS.results[c]["out"]
    return out


# revision 8
# speedup vs baseline: 1.6588x; 1.6588x over previous
"""Distributed multi-head attention kernel for one TRN2 chip (8 NeuronCores).

Problem: x[2,2048,1024] -> qkv -> 16-head attention -> out proj, f32 I/O.

Sharding: 8 cores = 2 batches x 4 head-groups (4 heads each).
Core c: batch b=c//4, head group g=c%4 (heads 4g..4g+3).
 - x fed host-transposed as xT[b] [1024,2048] bf16 (layout prep on host).
 - w_qkv column-sharded per head group. After attention, each core
   AllGathers its o^T [128,512] tiles (bf16) per (head-pair, q-tile) over
   its 4-core batch group, then projects with the FULL w_proj for its own
   q quarter (selected via a dynamic offset from the per-core qsel input)
   and writes output rows [512g:512g+512) of batch b directly.

Device algorithm (per core), all matmuls bf16 with f32 PSUM accumulate:
 1) qkT = w_qk.T @ xT   [512,2048]  (q pre-scaled by 1/sqrt(dh), bias via ACT)
 2) v_aug = xT.T @ w_v  [2048,4*65] (bias added on DVE; per-head ones column
    interleaved so the PV matmul also emits softmax denominators)
 3) per head-pair, per q-tile: sT = k @ qT, p = exp(sT) on ScalarE
    (PSUM->SBUF bf16; no max-subtraction needed: |scores/sqrt(dh)| < ~2),
    oT_aug += v_aug.T @ p (M=65: row 64 = denominator), normalize oT by
    1/denom during PSUM evict, DMA the tile to the AllGather bounce.
 4) o_full = AllGather(o^T) per (pair, q-tile); proj = o_full.T @ w_proj
    + b_proj for this core's q quarter; DMA out [512,1024].
"""

import os
import sys
import types
import numpy as np
import ml_dtypes

import concourse.bass as bass
import concourse.mybir as mybir
import concourse.bacc as bacc
import concourse.tile as tile
from concourse.bass_utils import run_bass_kernel_spmd

BF16 = mybir.dt.bfloat16
F32 = mybir.dt.float32
U32 = mybir.dt.uint32

B, N, D = 2, 2048, 1024
H, DH = 16, 64
SCALE = DH ** -0.5

P = 128                 # partitions
NT = 512                # token free-dim tile
KC = N // P             # 16 k-token chunks
QT = N // NT            # 4 q tiles
DC = D // P             # 8 d_model chunks
HPC = 4                 # heads per core
OF = HPC * DH           # 256 o-features per core
VW = HPC * (DH + 1)     # v_aug width (260): per head [v(64) | ones(1)]

CORE_IDS = list(range(8))
GROUPS = [[0, 1, 2, 3], [4, 5, 6, 7]]
LAST_RESULTS = None


def _install_ntff_shim():
    """Provide antenv.axon_hooks (absent from this image's antenv stub) so
    run_bass_kernel_spmd(trace=True) can reach the NTFF profiler in
    libaxon_pjrt.so. Only needed when profiling."""
    if "antenv.axon_hooks" in sys.modules:
        return
    try:
        from trn_agent_boot.trn_boot import _ntff_profile_via_ctypes
        hook = _ntff_profile_via_ctypes("/opt/axon/libaxon_pjrt.so")
    except Exception:
        hook = None
    mod = types.ModuleType("antenv.axon_hooks")
    mod._hook = hook
    mod.get_axon_ntff_profile_hook = lambda: mod._hook
    mod.set_axon_ntff_profile_hook = lambda h: setattr(mod, "_hook", h)
    sys.modules["antenv.axon_hooks"] = mod


def build_nc():
    nc = bacc.Bacc("TRN2", target_bir_lowering=False, debug=False, num_devices=8)

    xt_ext = nc.dram_tensor("xt", [D, N], BF16, kind="ExternalInput")
    wqk_ext = nc.dram_tensor("wqk", [D, 2 * OF], BF16, kind="ExternalInput")
    wv_ext = nc.dram_tensor("wv", [D, OF], BF16, kind="ExternalInput")
    bqk_ext = nc.dram_tensor("bqk", [P, 4], F32, kind="ExternalInput")
    bv_ext = nc.dram_tensor("bv", [1, OF], F32, kind="ExternalInput")
    wp_ext = nc.dram_tensor("wp", [D, D], BF16, kind="ExternalInput")
    bp_ext = nc.dram_tensor("bp", [1, D], F32, kind="ExternalInput")
    qsel_ext = nc.dram_tensor("qsel", [1, 1], U32, kind="ExternalInput")
    out_ext = nc.dram_tensor("out", [N // 4, D], F32, kind="ExternalOutput")

    # AllGather bounce buffers, blocked [qt][...] so every region is contiguous
    ag_in = [nc.dram_tensor(f"ag_in{p}", [QT, P, NT], BF16) for p in range(2)]
    ag_out = [nc.dram_tensor(f"ag_out{p}", [QT, 4 * P, NT], BF16) for p in range(2)]

    with tile.TileContext(nc) as tc:
        with (
            tc.tile_pool(name="xt_pool", bufs=1) as xt_pool,
            tc.tile_pool(name="w_pool", bufs=1) as w_pool,
            tc.tile_pool(name="qk_pool", bufs=1) as qk_pool,
            tc.tile_pool(name="v_pool", bufs=1) as v_pool,
            tc.tile_pool(name="o_pool", bufs=1) as o_pool,
            tc.tile_pool(name="const_pool", bufs=1) as const_pool,
        ):
            # ---- load inputs ----
            xt_t = []
            for k in range(DC):
                t = xt_pool.tile([P, N], BF16, name=f"xt{k}")
                nc.sync.dma_start(t[:], xt_ext[k * P:(k + 1) * P, :])
                xt_t.append(t)
            wqk_t = []
            for k in range(DC):
                t = w_pool.tile([P, 2 * OF], BF16, name=f"wqk{k}")
                nc.sync.dma_start(t[:], wqk_ext[k * P:(k + 1) * P, :])
                wqk_t.append(t)
            wv_t = []
            for k in range(DC):
                t = w_pool.tile([P, OF], BF16, name=f"wv{k}")
                nc.sync.dma_start(t[:], wv_ext[k * P:(k + 1) * P, :])
                wv_t.append(t)
            # w_proj chunk (p, g): rows 256g+128p .. +128 (AG row order)
            wp_t = {}
            for pair in range(2):
                for g in range(4):
                    t = w_pool.tile([P, D], BF16, name=f"wp{pair}{g}")
                    r0 = 256 * g + 128 * pair
                    nc.sync.dma_start(t[:], wp_ext[r0:r0 + P, :])
                    wp_t[(pair, g)] = t

            bqk_sb = const_pool.tile([P, 4], F32)
            nc.sync.dma_start(bqk_sb[:], bqk_ext[:])
            bv_row = const_pool.tile([1, OF], F32)
            nc.sync.dma_start(bv_row[:], bv_ext[:])
            bp_row = const_pool.tile([1, D], F32)
            nc.sync.dma_start(bp_row[:], bp_ext[:])

            bv_bc = const_pool.tile([P, OF], F32)
            nc.gpsimd.partition_broadcast(bv_bc[:], bv_row[:])
            bp_bc = const_pool.tile([P, D], F32)
            nc.gpsimd.partition_broadcast(bp_bc[:], bp_row[:])

            # ---- phase B: qkT = wqk.T @ xT  -> 4 tiles [128, 2048] bf16 ----
            # m=0: q heads 0-1, m=1: q heads 2-3, m=2: k heads 0-1, m=3: k 2-3
            qk_sb = [qk_pool.tile([P, N], BF16, name=f"qk{m}") for m in range(4)]
            with tc.tile_pool(name="ps_qk", bufs=4, space="PSUM") as ps_qk:
                for m in range(4):
                    for n in range(QT):
                        ps = ps_qk.tile([P, NT], F32)
                        for k in range(DC):
                            nc.tensor.matmul(
                                ps[:], wqk_t[k][:, m * P:(m + 1) * P],
                                xt_t[k][:, n * NT:(n + 1) * NT],
                                start=(k == 0), stop=(k == DC - 1))
                        nc.scalar.activation(
                            qk_sb[m][:, n * NT:(n + 1) * NT], ps[:],
                            mybir.ActivationFunctionType.Identity,
                            bias=bqk_sb[:, m:m + 1],
                            scale=SCALE if m < 2 else 1.0)

            # ---- phase C: v_aug [2048, 260] bf16 (16 tiles, ones interleaved) --
            v_sb = [v_pool.tile([P, VW], BF16, name=f"v{t}") for t in range(KC)]
            with tc.tile_pool(name="ps_v", bufs=4, space="PSUM") as ps_v:
                for t in range(KC):
                    ps = ps_v.tile([P, OF], F32)
                    for k in range(DC):
                        nc.tensor.matmul(
                            ps[:], xt_t[k][:, t * P:(t + 1) * P], wv_t[k][:],
                            start=(k == 0), stop=(k == DC - 1))
                    vdst = v_sb[t][:, :].rearrange("p (h c) -> p h c", c=DH + 1)
                    nc.vector.tensor_add(
                        vdst[:, :, 0:DH],
                        ps[:, :].rearrange("p (h c) -> p h c", c=DH),
                        bv_bc[:, :].rearrange("p (h c) -> p h c", c=DH))
                    nc.vector.memset(vdst[:, :, DH:DH + 1], 1.0)

            # ---- phase D: attention per head pair; AG per (pair, qtile) ----
            # o_pair[p] rows 0-63 = head 2p, rows 64-127 = head 2p+1
            o_pair = [o_pool.tile([P, N], BF16, name=f"o{p}") for p in range(2)]
            with (
                tc.tile_pool(name="ps_s", bufs=2, space="PSUM") as ps_s,
                tc.tile_pool(name="ps_o", bufs=2, space="PSUM") as ps_o,
                tc.tile_pool(name="pt_pool", bufs=3) as pt_pool,
                tc.tile_pool(name="nrm_pool", bufs=2) as nrm_pool,
            ):
                for p in range(2):
                    kt = qk_sb[2 + p]
                    qt_ = qk_sb[p]
                    hA, hB = 2 * p, 2 * p + 1
                    for qt in range(QT):
                        qs = slice(qt * NT, (qt + 1) * NT)
                        po0 = ps_o.tile([DH + 1, NT], F32, name="po0")
                        po1 = ps_o.tile([DH + 1, NT], F32, name="po1")
                        for c in range(KC):
                            cs = slice(c * P, (c + 1) * P)
                            s0 = ps_s.tile([P, NT], F32, name="s0")
                            s1 = ps_s.tile([P, NT], F32, name="s1")
                            nc.tensor.matmul(s0[:], kt[0:64, cs], qt_[0:64, qs],
                                             tile_position=(0, 0),
                                             start=True, stop=True)
                            nc.tensor.matmul(s1[:], kt[64:128, cs], qt_[64:128, qs],
                                             tile_position=(64, 0),
                                             start=True, stop=True)
                            pt0 = pt_pool.tile([P, NT], BF16, name="pt0")
                            pt1 = pt_pool.tile([P, NT], BF16, name="pt1")
                            nc.scalar.activation(
                                pt0[:], s0[:], mybir.ActivationFunctionType.Exp)
                            nc.scalar.activation(
                                pt1[:], s1[:], mybir.ActivationFunctionType.Exp)
                            st, sp = (c == 0), (c == KC - 1)
                            nc.tensor.matmul(
                                po0[:], v_sb[c][:, hA * (DH + 1):(hA + 1) * (DH + 1)],
                                pt0[:], start=st, stop=sp)
                            nc.tensor.matmul(
                                po1[:], v_sb[c][:, hB * (DH + 1):(hB + 1) * (DH + 1)],
                                pt1[:], start=st, stop=sp)
                        # normalize by 1/denominator (psum row 64, per q token)
                        dA = nrm_pool.tile([1, NT], F32, name="dA")
                        dB = nrm_pool.tile([1, NT], F32, name="dB")
                        nc.vector.tensor_copy(dA[0:1, :], po0[64:65, :])
                        nc.vector.tensor_copy(dB[0:1, :], po1[64:65, :])
                        rb = nrm_pool.tile([64, NT], F32, name="rb")
                        rb2 = nrm_pool.tile([64, NT], F32, name="rb2")
                        nc.gpsimd.partition_broadcast(rb[0:64, :], dA[0:1, :])
                        nc.gpsimd.partition_broadcast(rb2[0:64, :], dB[0:1, :])
                        rc = nrm_pool.tile([64, NT], F32, name="rc")
                        rc2 = nrm_pool.tile([64, NT], F32, name="rc2")
                        scr = nrm_pool.tile([64, NT], F32, name="scr")
                        nc.vector.reciprocal_approx_accurate(
                            rc[0:64, :], rb[0:64, :], scr[0:64, :])
                        nc.vector.reciprocal_approx_accurate(
                            rc2[0:64, :], rb2[0:64, :], scr[0:64, :])
                        nc.vector.tensor_mul(
                            o_pair[p][0:64, qs], po0[0:64, :], rc[0:64, :])
                        nc.vector.tensor_mul(
                            o_pair[p][64:128, qs], po1[0:64, :], rc2[0:64, :])
                        # ship this (pair, qtile) into the AllGather
                        nc.sync.dma_start(ag_in[p][qt, :, :], o_pair[p][:, qs])
                        nc.gpsimd.collective_compute(
                            "AllGather", mybir.AluOpType.bypass,
                            replica_groups=GROUPS,
                            ins=[ag_in[p][qt].opt()],
                            outs=[ag_out[p][qt].opt()])

            # ---- phase E: pull my q-quarter of o_full, proj, write out ----
            qsv = nc.sync.partition_id()  # unused; ensures pid tensor exists
            del qsv
            qreg = nc.sync.alloc_register("qsel_reg")
            nc.sync.reg_load(qreg, qsel_ext[0:1, 0:1])
            qsel = nc.sync.snap(qreg, donate=True, min_val=0, max_val=3)

            with (
                tc.tile_pool(name="of_pool", bufs=1) as of_pool,
                tc.tile_pool(name="ps_p", bufs=4, space="PSUM") as ps_p,
                tc.tile_pool(name="pr_pool", bufs=4) as pr_pool,
            ):
                ofull = {}
                for pair in range(2):
                    for g in range(4):
                        t = of_pool.tile([P, NT], BF16, name=f"of{pair}{g}")
                        nc.sync.dma_start(
                            t[:],
                            ag_out[pair][bass.ds(qsel, 1), g * P:(g + 1) * P, :])
                        ofull[(pair, g)] = t
                for qc in range(NT // P):
                    for o in range(2):
                        os_ = slice(o * NT, (o + 1) * NT)
                        ps = ps_p.tile([P, NT], F32)
                        first = True
                        for g in range(4):
                            for pair in range(2):
                                nc.tensor.matmul(
                                    ps[:],
                                    ofull[(pair, g)][:, qc * P:(qc + 1) * P],
                                    wp_t[(pair, g)][:, os_],
                                    start=first,
                                    stop=(g == 3 and pair == 1))
                                first = False
                        pr = pr_pool.tile([P, NT], F32)
                        nc.vector.tensor_add(pr[:], ps[:], bp_bc[:, os_])
                        nc.sync.dma_start(
                            out_ext[qc * P:(qc + 1) * P, os_], pr[:])

    nc.compile()
    return nc


_NC_CACHE = None


def _get_nc():
    global _NC_CACHE
    if _NC_CACHE is None:
        _NC_CACHE = build_nc()
    return _NC_CACHE


def _bf16(a):
    return np.ascontiguousarray(a.astype(ml_dtypes.bfloat16))


def kernel(x, w_qkv, b_qkv, w_proj, b_proj):
    global LAST_RESULTS
    x = np.asarray(x, dtype=np.float32)
    w_qkv = np.asarray(w_qkv, dtype=np.float32)
    b_qkv = np.asarray(b_qkv, dtype=np.float32)
    w_proj = np.asarray(w_proj, dtype=np.float32)
    b_proj = np.asarray(b_proj, dtype=np.float32)

    nc = _get_nc()

    in_maps = []
    for c in CORE_IDS:
        b, g = c // 4, c % 4
        cs = slice(g * OF, (g + 1) * OF)   # feature cols of this head group
        wq = w_qkv[:, 0 * D:1 * D][:, cs]
        wk = w_qkv[:, 1 * D:2 * D][:, cs]
        wv = w_qkv[:, 2 * D:3 * D][:, cs]
        bq = b_qkv[0 * D:1 * D][cs] * SCALE
        bk = b_qkv[1 * D:2 * D][cs]
        bqk = np.concatenate([bq, bk]).reshape(4, P).T.copy()  # [128, 4]
        in_maps.append({
            "xt": _bf16(x[b].T),
            "wqk": _bf16(np.concatenate([wq, wk], axis=1)),
            "wv": _bf16(wv),
            "bqk": np.ascontiguousarray(bqk, dtype=np.float32),
            "bv": np.ascontiguousarray(
                b_qkv[2 * D + g * OF:2 * D + (g + 1) * OF].reshape(1, OF)),
            "wp": _bf16(w_proj),
            "bp": np.ascontiguousarray(b_proj.reshape(1, D), dtype=np.float32),
            "qsel": np.array([[g]], dtype=np.uint32),
        })

    trace = bool(os.environ.get("KERNEL_TRACE"))
    if trace:
        _install_ntff_shim()
    LAST_RESULTS = run_bass_kernel_spmd(
        nc, in_maps, CORE_IDS, trace=trace)

    out = np.empty((B, N, D), dtype=np.float32)
    for c in CORE_IDS:
        b, g = c // 4, c % 4
        out[b, g * (N // 4):(g + 1) * (N // 4), :] = LAST_RESULTS.results[c]["out"]
    return out
